# revision 8
# baseline (speedup 1.0000x reference)
"""Causal dilated 1D conv (KW=4, dilation=8) as shifted matmuls on 8 TRN2 cores.

out[b,o,t] = sum_{k,c} W[o, c*4+k] * x[b, c, t + k*8 - 24]

Sharding: data-parallel over batch (16 batches -> 2 per core). Each core runs
an identical program: all weights stationary in SBUF, x streamed in 512-wide
time blocks (+24 halo, one combined 3D DMA per block), 16 accumulating
matmuls (4 c-chunks x 4 taps) per (out-chunk, time-block) PSUM group, PSUM
copied back via DVE and DMA'd out.

Inputs are cast to bf16 on host (fp32 PSUM accumulate): ~2.3e-3 on the
harness metric (max-abs-err / max|expected|, gate 2e-2). vs fp32r this
halves LDWEIGHTS to ~97ns, which then hides completely behind the 213.3ns
matmul stream: steady-state matmul start-to-start is ~216ns vs 232ns for
fp32r (whose fused fp32 weight reload costs ~19ns/MM and cannot be
amortized -- walrus emits one LDWEIGHTS per MATMUL regardless of repeated
stationary). 6 dummy matmuls on scratch SBUF warm the PE p-state
(0.65->2.4GHz takes ~3us of activity) while the first x/weight DMAs fly.

Measured ~458.6us vs a 436.9us PE-streaming floor (2048 MMs x 213.3ns).
Weight-tile DMA triggers alternate between the two HWDGE queues
(Sync/Act) so the 2MB bootstrap weight stream is not serialized behind
one queue.
The residue: ~8us fixed runtime preamble before DMA triggers issue, ~2.4us
first-tile DMA, ~9us of PE-sequencer 16KB instruction-fetch bubbles
(~430ns each; a For_i loop body stays resident and avoids them, but its
per-iteration all-engine barrier costs ~1.35us -- net loss), ~2us
DMA-bandwidth-gated bootstrap stalls, ~5.5us tail (last drain + teardown).

Tried and rejected: fp8 DoubleRow (plain e4m3 err ~5e-2 > gate; hi+lo
split needs 3 half-rate MMs = 1.5x), >512-wide matmul outputs (s3d3 ISA
limit), mixed bf16-stationary x f32r-moving (walrus verifier rejects),
per-oc 32KB weight slices and 128-col tail DMAs (256-512B DMA lines run
~20GB/s; keep per-partition DMA lines >= 1KB).
"""

import numpy as np

B = 16
C_IN = 512
C_OUT = 512
T = 8192
KW = 4
DIL = 8
PAD = (KW - 1) * DIL  # 24

N_CORES = 8
B_PER = B // N_CORES  # 2
P = 128
TBLK = 512
NT = T // TBLK        # 16
NCC = C_IN // P       # 4
NOC = C_OUT // P      # 4

_cache = {}


def _build():
    import concourse.tile as tile
    from concourse import bacc, mybir

    nc = bacc.Bacc("TRN2", target_bir_lowering=False, debug=False,
                   num_devices=N_CORES)
    bf16 = mybir.dt.bfloat16
    x = nc.dram_tensor("x", [B_PER, P, NCC, T + PAD], bf16,
                       kind="ExternalInput").ap()
    # weights pre-arranged on host as [cc, tap, c=128, o=512]
    wt = nc.dram_tensor("wt", [NCC, KW, P, C_OUT], bf16,
                        kind="ExternalInput").ap()
    out = nc.dram_tensor("out", [B_PER, C_OUT, T], mybir.dt.float32,
                         kind="ExternalOutput").ap()
    f32 = mybir.dt.float32
    f32r = bf16

    with tile.TileContext(nc) as tc:
        with tc.tile_pool(name="wpool", bufs=1) as wpool, \
             tc.tile_pool(name="xpool", bufs=8) as xpool, \
             tc.tile_pool(name="opool", bufs=8) as opool, \
             tc.tile_pool(name="pspool", bufs=8, space="PSUM") as pspool:

            XW = TBLK + PAD

            def load_xt(b, tb):
                xt = xpool.tile([P, NCC * XW], f32r, name="xt", tag="xt")
                nc.sync.dma_start(
                    xt[:].rearrange("p (a b) -> p a b", a=NCC),
                    x[b, :, :, tb * TBLK: tb * TBLK + XW])
                return xt

            # Warm the PE clock (p-state ramps to 2.4GHz after ~3us of
            # activity) with dummy matmuls on scratch SBUF while the first
            # x/weight DMAs are in flight. Sized to finish just before the
            # first real matmul's inputs land (~10us).
            warm = wpool.tile([P, 640], f32r, name="warm", tag="warm")
            nc.vector.memset(warm[:], 1.0)
            ps_warm = pspool.tile([P, TBLK], f32, name="ps", tag="ps")
            for _ in range(7):
                nc.tensor.matmul(ps_warm[:], warm[:, 0:P], warm[:, P:P + TBLK],
                                 start=True, stop=True)

            # Interleave the first time-block's per-cc x slices with their
            # matching weight tiles: the bootstrap fan-out consumes (cc=0,
            # k=0..3) first, so its inputs lead the wire stream and the PE
            # starts real matmuls ~10.4us in, fed continuously while the
            # remaining weights arrive.
            first_xt = xpool.tile([P, NCC * XW], f32r, name="xt", tag="xt")
            wtiles = [[None] * KW for _ in range(NCC)]
            for cc in range(NCC):
                nc.sync.dma_start(first_xt[:, cc * XW:(cc + 1) * XW],
                                  x[0, :, cc, 0:XW])
                for k in range(KW):
                    wtile = wpool.tile([P, C_OUT], f32r, name=f"w_{cc}_{k}",
                                       tag=f"w_{cc}_{k}")
                    # alternate the two HWDGE queues (Sync/Act) so weight
                    # triggers process in parallel during bootstrap
                    eng = nc.sync if (cc * KW + k) % 2 == 0 else nc.scalar
                    eng.dma_start(wtile[:], wt[cc, k])
                    wtiles[cc][k] = wtile

            n_acc = NCC * KW
            cks = [(cc, k) for cc in range(NCC) for k in range(KW)]

            # Bootstrap block: emit MMs in weight-DMA-arrival order, fanning
            # each arriving weight across the 4 oc PSUM banks, so the in-order
            # PE stream is never head-of-line blocked on a later weight tile.
            pss0 = [pspool.tile([P, TBLK], f32, name="ps", tag="ps")
                    for _ in range(NOC)]
            for ci, (cc, k) in enumerate(cks):
                for oc in range(NOC):
                    nc.tensor.matmul(
                        pss0[oc][:],
                        wtiles[cc][k][:, oc * P:(oc + 1) * P],
                        first_xt[:, cc * XW + k * DIL: cc * XW + k * DIL + TBLK],
                        start=(ci == 0),
                        stop=(ci == n_acc - 1),
                    )
            for oc in range(NOC):
                ot = opool.tile([P, TBLK], f32, name="ot", tag="ot")
                nc.vector.tensor_copy(ot[:], pss0[oc][:])
                nc.sync.dma_start(out[0, oc * P:(oc + 1) * P, 0:TBLK], ot[:])

            for b in range(B_PER):
                for tb in range(NT):
                    if b == 0 and tb == 0:
                        continue
                    last = (b == B_PER - 1 and tb == NT - 1)
                    xt = load_xt(b, tb)
                    for oc in range(NOC):
                        ps = pspool.tile([P, TBLK], f32, name="ps", tag="ps")
                        for ci, (cc, k) in enumerate(cks):
                            nc.tensor.matmul(
                                ps[:],
                                wtiles[cc][k][:, oc * P:(oc + 1) * P],
                                xt[:, cc * XW + k * DIL:
                                   cc * XW + k * DIL + TBLK],
                                start=(ci == 0),
                                stop=(ci == n_acc - 1),
                            )
                        if last and oc == NOC - 1:
                            # 2x256-col tail chunks (1KB DMA lines) so only
                            # ~1us trails the final matmul.
                            for q in range(2):
                                otq = opool.tile([P, TBLK], f32,
                                                 name="ot", tag="ot")
                                nc.vector.tensor_copy(
                                    otq[:, 0:256], ps[:, q * 256:(q + 1) * 256])
                                nc.sync.dma_start(
                                    out[b, oc * P:(oc + 1) * P,
                                        tb * TBLK + q * 256:
                                        tb * TBLK + (q + 1) * 256],
                                    otq[:, 0:256])
                        else:
                            ot = opool.tile([P, TBLK], f32, name="ot", tag="ot")
                            nc.vector.tensor_copy(ot[:], ps[:])
                            nc.sync.dma_start(
                                out[b, oc * P:(oc + 1) * P,
                                    tb * TBLK:(tb + 1) * TBLK],
                                ot[:])

    nc.compile()
    return nc


def _get_nc():
    if "nc" not in _cache:
        _cache["nc"] = _build()
    return _cache["nc"]


def _make_in_maps(x, W):
    import ml_dtypes
    bf16 = ml_dtypes.bfloat16
    xpad = np.pad(np.ascontiguousarray(x, dtype=np.float32),
                  ((0, 0), (0, 0), (PAD, 0))).astype(bf16)
    # [B, C_IN, T+PAD] -> [B, P, NCC, T+PAD] with c = cc*128 + p
    xpad = np.ascontiguousarray(
        xpad.reshape(B, NCC, P, T + PAD).transpose(0, 2, 1, 3))
    w = np.ascontiguousarray(W, dtype=np.float32).reshape(C_OUT, C_IN, KW)
    # wt[cc, k, c, o] = W[o, (cc*128+c)*KW + k]
    wt = np.transpose(w.reshape(C_OUT, NCC, P, KW),
                      (1, 3, 2, 0)).astype(bf16).copy()
    return [{"x": np.ascontiguousarray(xpad[i * B_PER:(i + 1) * B_PER]),
             "wt": wt} for i in range(N_CORES)]


def kernel(x, W):
    from concourse.bass_utils import run_bass_kernel_spmd

    nc = _get_nc()
    in_maps = _make_in_maps(x, W)
    res = run_bass_kernel_spmd(nc, in_maps, list(range(N_CORES)))
    return np.concatenate([r["out"] for r in res.results], axis=0)

